# revision 5
# baseline (speedup 1.0000x reference)
"""DeepSeek-V3-style MoE kernel for Trainium2, 8-core expert-parallel.

Strategy (v3):
  - Routing runs on HOST in f32 (exactly mirrors the reference), producing
    dense combine weights cw [T, E]. The host performs the all-to-all token
    dispatch: for each core it gathers, pads and TRANSPOSES the selected
    token rows into xgt [H, CTOT] (bf16), so the device needs no on-device
    routing, no gather, and no PE transposes at all.
  - Experts are load-balanced: sort experts by token count (desc), slot j
    holds ranks [8j, 8j+8), one per core. All cores share one static cap
    per slot (SPMD requires identical shapes); caps are 32-granular and
    tight since ranks within an octile have similar counts.
  - Device = pure bf16 GEMM pipeline (fp32 PSUM accumulate):
      shared expert tensor-parallel over IS (2048 -> 256 per core) over all
      T tokens, then 8 routed expert slots. Weight-stationary matmuls:
      h1T/h3T [I, tok] = w @ xT, silu*mul on Act/DVE, down-proj back to
      [tok, H] with actT as stationary operand. Combine weight applied as a
      per-partition scalar on the PSUM->SBUF copy of y. Expert weights are
      software-pipelined two slots ahead.
  - Outputs are bf16: shared partial [T, H] per core (host sums 8) and
    routed yout [CTOT, H] per core (host adds per-expert slices into the
    output; token lists within one expert are unique so vectorized
    fancy-index += is safe).
"""

import sys
import numpy as np

sys.path.insert(0, "/opt/trn_rl_repo")

T, H, E, I, IS = 4096, 1024, 64, 512, 2048
N_GROUP, TOPK_GROUP, TOP_K = 8, 4, 8
ROUTED_SCALE = 2.5

NCORES = 8
EL = E // NCORES          # expert slots per core
ISL = IS // NCORES        # shared intermediate slice per core
HB = H // 128             # 8
IB = I // 128             # 4
ISB = ISL // 128          # 2
NH = H // 512             # 2 (psum-bank halves of the down-proj)
TCH = 512                 # token chunk (psum bank limit, f32)
SCHUNKS = [256, 256] + [512] * 7   # shared-phase chunk plan (sum == T)


def build_kernel(caps):
    from concourse import bacc, mybir, tile

    f32 = mybir.dt.float32
    bf = mybir.dt.bfloat16
    AF = mybir.ActivationFunctionType
    OP = mybir.AluOpType

    CTOT = sum(caps)
    ncols = [-(-c // 128) for c in caps]          # cw columns per slot
    CBT = sum(ncols)

    nc = bacc.Bacc("TRN2", target_bir_lowering=False, debug=False,
                   num_devices=NCORES)

    hidT = nc.declare_dram_parameter("hidT", [H, T], bf, isOutput=False)
    xgt = nc.declare_dram_parameter("xgt", [H, CTOT], bf, isOutput=False)
    cwc = nc.declare_dram_parameter("cwc", [128, CBT], f32, isOutput=False)
    w13t = nc.declare_dram_parameter("w13t", [EL, H, 2 * I], bf,
                                     isOutput=False)
    w2t = nc.declare_dram_parameter("w2t", [EL, I, H], bf, isOutput=False)
    ws13t = nc.declare_dram_parameter("ws13t", [H, 2 * ISL], bf,
                                      isOutput=False)
    ws2t = nc.declare_dram_parameter("ws2t", [ISL, H], bf, isOutput=False)
    outs = nc.declare_dram_parameter("outs", [T, H], bf, isOutput=True)
    yout = nc.declare_dram_parameter("yout", [CTOT, H], bf, isOutput=True)

    import contextlib
    with tile.TileContext(nc) as tc, contextlib.ExitStack() as ctx:
        p_const = ctx.enter_context(tc.tile_pool(name="const", bufs=1))
        p_w = ctx.enter_context(tc.tile_pool(name="w", bufs=3))
        p_x = ctx.enter_context(tc.tile_pool(name="x", bufs=3))
        p_act = ctx.enter_context(tc.tile_pool(name="act", bufs=2))
        p_y = ctx.enter_context(tc.tile_pool(name="y", bufs=3))
        ps_h = ctx.enter_context(tc.tile_pool(name="ps_h", bufs=2,
                                              space="PSUM"))
        ps_y = ctx.enter_context(tc.tile_pool(name="ps_y", bufs=2,
                                              space="PSUM"))

        # shared-expert weights + combine weights, resident.
        # ws13 is loaded in column blocks ordered so the first gated_block
        # (isb=0: ws1 cols [0:128), ws3 cols [256:384)) can start earliest.
        ws13_sb = p_const.tile([128, HB, 2 * ISL], bf, tag="ws13")
        for cb in (0, 2, 1, 3):
            nc.sync.dma_start(
                out=ws13_sb[:, :, cb * 128:(cb + 1) * 128],
                in_=ws13t[:, cb * 128:(cb + 1) * 128].rearrange(
                    "(b p) i -> p b i", p=128))
        ws2_sb = p_const.tile([128, ISB, H], bf, tag="ws2")
        nc.sync.dma_start(out=ws2_sb[:],
                          in_=ws2t[:].rearrange("(b p) i -> p b i", p=128))
        cw_sb = p_const.tile([128, CBT], f32, tag="cw")
        nc.sync.dma_start(out=cw_sb[:], in_=cwc[:])

        def load_w(j):
            w13sb = p_w.tile([128, HB, 2 * I], bf, tag="w13")
            nc.sync.dma_start(
                out=w13sb[:],
                in_=w13t[j].rearrange("(b p) i -> p b i", p=128))
            w2sb = p_w.tile([128, IB, H], bf, tag="w2")
            nc.sync.dma_start(
                out=w2sb[:],
                in_=w2t[j].rearrange("(b p) i -> p b i", p=128))
            return w13sb, w2sb

        def gated_block(xT, wsb, nI, Nc, act_tag):
            """h1T/h3T -> silu*mul -> actT [128, nI-blocks, Nc] bf16."""
            actT = p_act.tile([128, nI, TCH], bf, tag=act_tag)
            for ib in range(nI):
                h1 = ps_h.tile([128, TCH], f32, tag="h1")
                h3 = ps_h.tile([128, TCH], f32, tag="h3")
                for hb in range(HB):
                    nc.tensor.matmul(
                        out=h1[:, :Nc],
                        lhsT=wsb[:, hb, ib * 128:(ib + 1) * 128],
                        rhs=xT[:, hb, :Nc],
                        start=(hb == 0), stop=(hb == HB - 1))
                for hb in range(HB):
                    nc.tensor.matmul(
                        out=h3[:, :Nc],
                        lhsT=wsb[:, hb, nI * 128 + ib * 128:
                                 nI * 128 + (ib + 1) * 128],
                        rhs=xT[:, hb, :Nc],
                        start=(hb == 0), stop=(hb == HB - 1))
                sil = p_act.tile([128, TCH], f32, tag="sil")
                nc.scalar.activation(sil[:, :Nc], h1[:, :Nc], AF.Silu)
                nc.vector.tensor_tensor(out=actT[:, ib, :Nc],
                                        in0=sil[:, :Nc], in1=h3[:, :Nc],
                                        op=OP.mult)
            return actT

        def down_proj(actT, wsb, nI, tb, r, cwap):
            """y [r, H] from actT cols [tb, tb+r); cwap None or [r,1]."""
            ysb = p_y.tile([128, H], bf, tag="ysb")
            for nh in range(NH):
                y = ps_y.tile([128, 512], f32, tag=f"y{nh}")
                for ib in range(nI):
                    nc.tensor.matmul(
                        out=y[:r, :],
                        lhsT=actT[:, ib, tb:tb + r],
                        rhs=wsb[:, ib, nh * 512:(nh + 1) * 512],
                        start=(ib == 0), stop=(ib == nI - 1))
                if cwap is None:
                    nc.any.tensor_copy(
                        out=ysb[:r, nh * 512:(nh + 1) * 512], in_=y[:r, :])
                else:
                    nc.vector.tensor_scalar_mul(
                        ysb[:r, nh * 512:(nh + 1) * 512], y[:r, :], cwap)
            return ysb

        # ---------------- shared expert over all T tokens ----------------
        # Routed-slot weights are prefetched AFTER the first chunks' DMAs so
        # they don't block the critical startup transfers (in-order queue).
        wq = [None, None]
        c0 = 0
        for ci, Nc in enumerate(SCHUNKS):
            hT = p_x.tile([128, HB, TCH], bf, tag="xT")
            nc.sync.dma_start(
                out=hT[:, :, :Nc],
                in_=hidT[:, c0:c0 + Nc].rearrange("(b p) t -> p b t", p=128))
            actT = gated_block(hT, ws13_sb, ISB, Nc, "actS")
            for tb in range(0, Nc, 128):
                ysb = down_proj(actT, ws2_sb, ISB, tb, 128, None)
                r0 = c0 + tb
                nc.sync.dma_start(out=outs[r0:r0 + 128, :], in_=ysb[:])
            c0 += Nc
            if ci < 2:
                wq[ci] = load_w(ci)

        # ---------------- routed experts ----------------
        off = 0
        cwoff = 0
        for j in range(EL):
            Cj = caps[j]
            if Cj == 0:
                continue
            w13sb, w2sb = wq[j % 2]
            if j + 2 < EL and caps[j + 2] > 0:
                wq[j % 2] = load_w(j + 2)
            for cc in range(0, Cj, TCH):
                Nc = min(TCH, Cj - cc)
                xT = p_x.tile([128, HB, TCH], bf, tag="xT")
                nc.sync.dma_start(
                    out=xT[:, :, :Nc],
                    in_=xgt[:, off + cc:off + cc + Nc].rearrange(
                        "(b p) t -> p b t", p=128))
                actT = gated_block(xT, w13sb, IB, Nc, "actR")
                for tb in range(0, Nc, 128):
                    r = min(128, Nc - tb)
                    gb = cwoff + (cc + tb) // 128
                    ysb = down_proj(actT, w2sb, IB, tb, r,
                                    cw_sb[:r, gb:gb + 1])
                    r0 = off + cc + tb
                    nc.sync.dma_start(out=yout[r0:r0 + r, :], in_=ysb[:r, :])
            off += Cj
            cwoff += -(-Cj // 128)

    nc.compile()
    return nc


_CACHE = {}


def _bf16(x):
    """Fast f32 -> bf16 (round to nearest even) via bit manipulation."""
    import ml_dtypes
    v = np.ascontiguousarray(x, dtype=np.float32).view(np.uint32)
    r = ((v + 0x7FFF + ((v >> 16) & 1)) >> 16).astype(np.uint16)
    return r.view(ml_dtypes.bfloat16)


def _np_route(hidden, gate_w, e_bias):
    """f32 numpy clone of the reference routing; returns dense cw [T, E]."""
    logits = (hidden @ gate_w.T).astype(np.float32)
    scores = (1.0 / (1.0 + np.exp(-logits))).astype(np.float32)
    swb = (scores + e_bias[None, :]).astype(np.float32)
    g = swb.reshape(T, N_GROUP, E // N_GROUP)
    gs = np.sort(g, axis=-1)[:, :, -2:].sum(-1, dtype=np.float32)
    thr_g = np.sort(gs, axis=-1)[:, -TOPK_GROUP:-TOPK_GROUP + 1]
    gmask = (gs >= thr_g).astype(np.float32)
    mswb = swb * np.repeat(gmask, E // N_GROUP, axis=-1)
    thr = np.sort(mswb, axis=-1)[:, -TOP_K:-TOP_K + 1]
    nmask = (mswb >= thr).astype(np.float32)
    s = scores * nmask
    s = s / (s.sum(-1, keepdims=True) + 1e-20) * ROUTED_SCALE
    return s


def _plan(inputs):
    """Routing + expert->(core, slot) assignment + static per-slot caps."""
    hidden = np.asarray(inputs["hidden_states"], dtype=np.float32)
    gate_w = np.asarray(inputs["gate_w"], dtype=np.float32)
    e_bias = np.asarray(inputs["e_bias"], dtype=np.float32)
    cw = _np_route(hidden, gate_w, e_bias)
    counts = (cw > 0).sum(0)                       # [E]
    order = np.argsort(-counts, kind="stable")
    assign = order.reshape(EL, NCORES)             # [slot, core] -> expert
    caps = tuple(int(-(-int(counts[assign[j]].max()) // 32) * 32)
                 for j in range(EL))
    return cw, assign, caps


def _host_prep(inputs, cw, assign, caps):
    hidden = np.asarray(inputs["hidden_states"], dtype=np.float32)
    w1 = np.asarray(inputs["w1"], dtype=np.float32)
    w2 = np.asarray(inputs["w2"], dtype=np.float32)
    w3 = np.asarray(inputs["w3"], dtype=np.float32)
    ws1 = np.asarray(inputs["ws1"], dtype=np.float32)
    ws2 = np.asarray(inputs["ws2"], dtype=np.float32)
    ws3 = np.asarray(inputs["ws3"], dtype=np.float32)

    CTOT = sum(caps)
    ncols = [-(-c // 128) for c in caps]
    CBT = sum(ncols)
    hidT = _bf16(hidden.T)

    # global weight prep (transposed, bf16), sliced per core afterwards
    w13_all = _bf16(np.concatenate(
        [w1.transpose(0, 2, 1), w3.transpose(0, 2, 1)], axis=2))  # [E,H,2I]
    w2t_all = _bf16(w2.transpose(0, 2, 1))                        # [E,I,H]

    in_maps = []
    tok_lists = []
    for k in range(NCORES):
        isl = slice(k * ISL, (k + 1) * ISL)
        X = np.zeros((CTOT, H), dtype=np.float32)
        cwpad = np.zeros(CBT * 128, dtype=np.float32)
        toks_k = []
        offv = 0
        offc = 0
        for j in range(EL):
            e = assign[j, k]
            tk = np.nonzero(cw[:, e] > 0)[0]
            n = len(tk)
            X[offv:offv + n] = hidden[tk]
            cwpad[offc:offc + n] = cw[tk, e]
            toks_k.append((offv, tk))
            offv += caps[j]
            offc += ncols[j] * 128
        es = assign[:, k]
        ws13 = np.concatenate([ws1[isl].T, ws3[isl].T], axis=1)  # [H, 2ISL]
        in_maps.append({
            "hidT": hidT,
            "xgt": _bf16(X.T),
            "cwc": np.ascontiguousarray(cwpad.reshape(CBT, 128).T),
            "w13t": np.ascontiguousarray(w13_all[es]),
            "w2t": np.ascontiguousarray(w2t_all[es]),
            "ws13t": _bf16(ws13),
            "ws2t": _bf16(ws2[:, isl].T),
        })
        tok_lists.append(toks_k)
    return in_maps, tok_lists


def kernel(**inputs) -> np.ndarray:
    from concourse.bass_utils import run_bass_kernel_spmd

    cw, assign, caps = _plan(inputs)
    if caps not in _CACHE:
        _CACHE[caps] = build_kernel(caps)
    nc = _CACHE[caps]
    in_maps, tok_lists = _host_prep(inputs, cw, assign, caps)
    res = run_bass_kernel_spmd(nc, in_maps, list(range(NCORES)))
    out = np.zeros((T, H), dtype=np.float32)
    for k in range(NCORES):
        out += res.results[k]["outs"].astype(np.float32)
    for k in range(NCORES):
        yk = res.results[k]["yout"]
        for offv, tk in tok_lists[k]:
            if len(tk):
                out[tk] += yk[offv:offv + len(tk)].astype(np.float32)
    return out
